# revision 34
# baseline (speedup 1.0000x reference)
"""Trainium2 Bass kernel for nn_MultiHeadAttention (B=2, S=2048, D=1024, H=16, causal).

Sharding across 8 NeuronCores -- NO on-device collective:
  - Core c owns batch b=c//4 and head-group g=c%4 (4 heads).  Wq/Wk/Wv are
    column-sharded (256 features per core), Wo is row-sharded; each core
    emits a PARTIAL output projection over the full 2048 tokens and the
    host sums the 4 partials per batch at unshard time.  This removes the
    AllGather + barrier that cost ~190us in the token-sharded design.
  - Everything on-chip is bf16 (PSUM accumulation fp32); the softmax
    scale is folded into Wk on the host.  The K bias is dropped (it adds
    a per-query constant to scores, which softmax cancels); the V bias is
    dropped on-device (attention rows sum to 1, so it becomes the
    constant bv @ Wo.T folded into bo on the host); the Q bias is a
    per-partition tensor_scalar add fused into the PSUM->SBUF copy.
  - Pipelined per 512-token chunk tc: project K/V/Q for chunk tc, run
    attention for query chunk tc (keys 0..512*tc+511), project chunk tc+1
    BEFORE normalize/output-projection of tc so the softmax-normalize
    tail never stalls the PE.
  - Attention per head pair (feature block = 128 partitions): score
    matmuls for the two heads are row-tiled (partitions 0:64 / 64:128)
    into one 2-bank PSUM tile so they run concurrently; one wide exp
    covers both heads; ONE flat software pipeline over (pair, block) --
    the ctx matmuls of block b issue after the scores of block b+1,
    across the pair boundary too, so the PE never waits on the scalar
    engine's exp.  Diagonal key blocks are width-trimmed to 512-128*o
    columns; the causal mask restricted to the computed window is the
    same [128,2,128] staircase for every block (beyond 128 columns it is
    all-ones).  ctx accumulates in PSUM across key blocks with a 65th
    stationary V column (= softmax denominator); ctx is copied raw
    (bf16) to SBUF immediately to free the PSUM banks, and the
    normalize chain -- recip = exp(-ln(d)) on scalar, replicated across
    64 partitions by gpsimd partition_broadcast (the PE and DVE stay
    untouched), DVE mult -- runs off the critical path.  Odd heads
    reach partitions 64:128 of ctx_sb via one small SBUF->SBUF DMA per
    (chunk, pair).  On the LAST chunk, pair0 is normalized eagerly
    (during pair1's attention) so only pair1's chain trails the PE.
  - PSUM: ps_a 2x[128,1024] double-buffers score tiles; ps_b 4x[128,512]
    carries ctx accumulators, projection and output-projection tiles in
    one FIFO rotation.
  - All DMAs issue from the sync/scalar hardware DGE queues (gpsimd's
    software DGE adds ~5us trigger latency); the startup-critical wk +
    x-chunk-0 bytes go first on separate queues so the x bulk never
    competes with them.
"""
import numpy as np
import ml_dtypes

import concourse.bass as bass
import concourse.bacc as bacc
import concourse.mybir as mybir
import concourse.tile as tile
from concourse.bass_utils import run_bass_kernel_spmd

B, S, D, H, HD = 2, 2048, 1024, 16, 64
NC = 8
P = 128
F32 = mybir.dt.float32
BF = mybir.dt.bfloat16
NPBF = ml_dtypes.bfloat16

TRACE = False        # set True (e.g. from test.py) to capture an NTFF profile
LAST_RESULT = None   # BassKernelResults of the most recent kernel() call

_ACT_PATCHED = False


def _patch_act_tables():
    """Steer Bacc's act-table-load pass to the combined natural_log+exp
    set so a kernel using both Exp and Ln takes ONE table load instead of
    alternating between table sets (~1.3us per switch on scalar)."""
    global _ACT_PATCHED
    if _ACT_PATCHED:
        return
    import concourse.bacc as _bacc
    _orig = _bacc.get_activation_tables

    def _filtered(arch):
        t = _orig(arch)
        fexp = mybir.ActivationFunctionType.Exp
        fln = mybir.ActivationFunctionType.Ln
        out = {}
        for name, fns in t.items():
            if name != "natural_log_exp_and_others" and (
                    fexp in fns or fln in fns):
                fns = fns - {fexp, fln}
            out[name] = fns
        return out

    _bacc.get_activation_tables = _filtered
    _ACT_PATCHED = True


def _emit(causal: bool):
    nc = bacc.Bacc(trn_type="TRN2", num_devices=NC)
    fexp = mybir.ActivationFunctionType.Exp
    fln = mybir.ActivationFunctionType.Ln
    _patch_act_tables()

    xT = nc.dram_tensor("xT", [D, S], BF, kind="ExternalInput")
    wqT = nc.dram_tensor("wqT", [D, 256], BF, kind="ExternalInput")
    wkT = nc.dram_tensor("wkT", [D, 256], BF, kind="ExternalInput")
    wvT = nc.dram_tensor("wvT", [D, 256], BF, kind="ExternalInput")
    woT = nc.dram_tensor("woT", [256, D], BF, kind="ExternalInput")
    bqc_d = nc.dram_tensor("bqc", [P, 2], F32, kind="ExternalInput")
    if causal:
        cm_d = nc.dram_tensor("cm", [P, 2, P], BF, kind="ExternalInput")
    outT = nc.dram_tensor("outT", [D, S], BF, kind="ExternalOutput")

    with tile.TileContext(nc) as tc, \
         tc.tile_pool(name="const", bufs=1) as const, \
         tc.tile_pool(name="big", bufs=1) as big, \
         tc.tile_pool(name="oio", bufs=3) as oio, \
         tc.tile_pool(name="ex", bufs=8) as ex, \
         tc.tile_pool(name="u", bufs=4) as up, \
         tc.tile_pool(name="sm", bufs=2) as sm, \
         tc.tile_pool(name="ps_a", bufs=2, space="PSUM") as ps_a, \
         tc.tile_pool(name="ps_b", bufs=4, space="PSUM") as ps_b:

        # ---------- constants / inputs ----------
        ones = const.tile([1, P], BF)
        nc.gpsimd.memset(ones[:], 1.0)
        bqc_sb = const.tile([P, 2], F32)
        nc.scalar.dma_start(bqc_sb[:], bqc_d[:])
        if causal:
            cm_sb = const.tile([P, 2, P], BF)
            nc.scalar.dma_start(cm_sb[:], cm_d[:])

        wk_sb = big.tile([P, 8, 256], BF)
        wv_sb = big.tile([P, 8, 256], BF)
        wq_sb = big.tile([P, 8, 256], BF)
        wo_sb = big.tile([P, 2, D], BF)
        xt_sb = big.tile([P, 8, S], BF)
        kt_sb = big.tile([P, 2, S], BF)
        qt_sb = big.tile([P, 2, S], BF)
        v_sb = big.tile([P, 16, 4, 65], BF)
        ctx_sb = big.tile([P, 2, S], BF)
        nc.gpsimd.memset(v_sb[:, :, :, 64:65], 1.0)

        wkr = wkT.rearrange("(o p) f -> p o f", p=P)
        wvr = wvT.rearrange("(o p) f -> p o f", p=P)
        wqr = wqT.rearrange("(o p) f -> p o f", p=P)
        wor = woT.rearrange("(o p) f -> p o f", p=P)
        xr = xT.rearrange("(o p) t -> p o t", p=P)
        outr = outT.rearrange("(o p) t -> p o t", p=P)

        # Input DMAs: the critical path is wk + x chunk 0 (feeds the
        # first K matmuls).  Only sync/scalar issue DMAs (gpsimd DMAs go
        # through the slow software DGE).  scalar: consts then x chunk 0;
        # sync: weights then the bulk of x, so the bulk never competes
        # with the startup-critical bytes.
        nc.sync.dma_start(wk_sb[:], wkr[:])
        nc.sync.dma_start(wq_sb[:], wqr[:])
        nc.sync.dma_start(wv_sb[:], wvr[:])
        for kt in range(8):
            nc.scalar.dma_start(xt_sb[:, kt, 0:512], xr[:, kt, 0:512])
        for kt in range(8):
            nc.sync.dma_start(xt_sb[:, kt, 512:2048], xr[:, kt, 512:2048])
        nc.sync.dma_start(wo_sb[:], wor[:])

        def proj_chunk(tc_i):
            t0 = 512 * tc_i
            # K^T and Q^T: out[feat, tok], feature block fb == head pair.
            # Q bias is a per-partition scalar add fused into the copy.
            # K bias is DROPPED: it adds a per-query constant to every
            # score column, which the softmax normalization cancels.
            for fb in range(2):
                pt = ps_b.tile([P, 512], F32, tag="psb")
                for kt in range(8):
                    nc.tensor.matmul(
                        pt[:], wk_sb[:, kt, 128 * fb:128 * fb + 128],
                        xt_sb[:, kt, t0:t0 + 512],
                        start=(kt == 0), stop=(kt == 7))
                nc.vector.tensor_copy(kt_sb[:, fb, t0:t0 + 512], pt[:])
            for fb in range(2):
                pt = ps_b.tile([P, 512], F32, tag="psb")
                for kt in range(8):
                    nc.tensor.matmul(
                        pt[:], wq_sb[:, kt, 128 * fb:128 * fb + 128],
                        xt_sb[:, kt, t0:t0 + 512],
                        start=(kt == 0), stop=(kt == 7))
                nc.vector.tensor_scalar_add(
                    qt_sb[:, fb, t0:t0 + 512], pt[:], bqc_sb[:, fb:fb + 1])
            # V: out[tok, feat] per 128-token block (65th col pre-set to
            # 1).  V bias is DROPPED: attention rows sum to 1 after
            # normalization, so it shifts the output by the constant
            # bv @ Wo.T which the host folds into bo.
            for tb in range(4):
                jb = 4 * tc_i + tb
                pt = ps_b.tile([P, 512], F32, tag="psb")
                for kt in range(8):
                    nc.tensor.matmul(
                        pt[:, 0:256],
                        xt_sb[:, kt, t0 + 128 * tb:t0 + 128 * tb + 128],
                        wv_sb[:, kt, :], start=(kt == 0), stop=(kt == 7))
                nc.vector.tensor_copy(
                    v_sb[:, jb, :, 0:64],
                    pt[:, 0:256].rearrange("p (h d) -> p h d", h=4))

        def attn_chunk(tc_i, eager_norm=None):
            """Scores+exp+ctx for both head pairs.  One flat software
            pipeline over blocks (pair-major): the ctx matmuls of block b
            issue after the scores of block b+1, across the pair boundary
            too, so the PE never waits on the scalar engine's exp."""
            t0 = 512 * tc_i
            jn = 4 * tc_i + 4 if causal else 16
            ctx = {}
            prev = None
            us = []

            def emit_ctx(pair, pj, pet, pqo, pwid):
                for hh in range(2):
                    nc.tensor.matmul(
                        ctx[pair][hh][0:65, pqo:pqo + pwid],
                        v_sb[:, pj, 2 * pair + hh, :], pet[:, hh, 0:pwid],
                        start=(pj == 0), stop=(pj == jn - 1))
                if pj == jn - 1:
                    for hh in range(2):
                        u = up.tile([65, 512], BF, tag="u")
                        nc.vector.tensor_copy(u[:], ctx[pair][hh][0:65, :])
                        us.append(u)
                    if eager_norm is not None and pair == 0:
                        eager_norm(us[0], us[1])

            for pair in range(2):
                c0 = ps_b.tile([P, 512], F32, tag="psb")
                c1 = ps_b.tile([P, 512], F32, tag="psb")
                ctx[pair] = (c0, c1)
                for j in range(jn):
                    o_ = j - 4 * tc_i if causal else -1
                    qo = 0 if o_ < 0 else 128 * o_
                    wid = 512 - qo
                    sc = ps_a.tile([P, 1024], F32, tag="psa")
                    for hh in range(2):
                        nc.tensor.matmul(
                            sc[:, 512 * hh:512 * hh + wid],
                            kt_sb[64 * hh:64 * hh + 64, pair,
                                  128 * j:128 * j + 128],
                            qt_sb[64 * hh:64 * hh + 64, pair,
                                  t0 + qo:t0 + qo + wid],
                            start=True, stop=True)
                    et = ex.tile([P, 2, 512], BF, tag="exp")
                    if wid == 512:
                        nc.scalar.activation(et[:, :, :], sc[:, :], fexp)
                    else:
                        nc.scalar.activation(
                            et[:, :, 0:wid],
                            sc[:].rearrange("p (s n) -> p s n", s=2)
                            [:, :, 0:wid], fexp)
                    if o_ >= 0:
                        nc.vector.tensor_tensor(
                            et[:, :, 0:P], et[:, :, 0:P], cm_sb[:],
                            mybir.AluOpType.mult)
                    if prev is not None:
                        emit_ctx(*prev)
                    prev = (pair, j, et, qo, wid)
            emit_ctx(*prev)
            return us

        def norm_chunk(tc_i, pairs, on_pe=False, eager=False):
            """recip = exp(-ln(denominator)) for both heads of a pair into
            one [1,1024] tile; ONE rank-1 matmul replicates it to a
            [64,1024] PSUM region that the DVE mults read directly (no
            PSUM->SBUF copy).  Even heads multiply straight into
            ctx_sb[0:64]; odd heads go via a [64,2,512] staging tile and
            ONE SBUF->SBUF DMA per chunk to partitions 64:128."""
            t0 = 512 * tc_i
            for pair, u0, u1 in pairs:
                rcp2 = sm.tile([1, 1024], BF, tag="rcp")
                lnd0 = sm.tile([1, 512], F32, tag="lnd")
                nc.scalar.activation(lnd0[:], u0[64:65, 0:512], fln)
                nc.scalar.activation(rcp2[0:1, 0:512], lnd0[:], fexp,
                                     scale=-1.0)
                lnd1 = sm.tile([1, 512], F32, tag="lnd")
                nc.scalar.activation(lnd1[:], u1[64:65, 0:512], fln)
                nc.scalar.activation(rcp2[0:1, 512:1024], lnd1[:], fexp,
                                     scale=-1.0)
                if on_pe:
                    rep_ps = ps_b.tile([P, 512], F32, tag="psb")
                    nc.tensor.matmul(rep_ps[0:64, :], ones[0:1, 0:64],
                                     rcp2[0:1, 0:512], start=True, stop=True)
                    rep_ps2 = ps_b.tile([P, 512], F32, tag="psb")
                    nc.tensor.matmul(rep_ps2[0:64, :], ones[0:1, 0:64],
                                     rcp2[0:1, 512:1024],
                                     start=True, stop=True)
                    r0, r1 = rep_ps[0:64, :], rep_ps2[0:64, :]
                else:
                    rep = sm.tile([64, 1024], BF, tag="rep")
                    nc.gpsimd.partition_broadcast(rep[:], rcp2[0:1, :])
                    r0, r1 = rep[:, 0:512], rep[:, 512:1024]
                # eager (mid-attention) norms multiply on gpsimd so the
                # DVE queue stays clear for the mask mults / u copies the
                # attention pipeline depends on
                veng = nc.gpsimd if eager else nc.vector
                veng.tensor_tensor(
                    ctx_sb[0:64, pair, t0:t0 + 512], u0[0:64, :],
                    r0, mybir.AluOpType.mult)
                ctmp = sm.tile([64, 512], BF, tag="ctmp")
                veng.tensor_tensor(
                    ctmp[:], u1[0:64, :],
                    r1, mybir.AluOpType.mult)
                # per-pair DMA: pair0's transfer overlaps pair1's norm
                nc.scalar.dma_start(ctx_sb[64:128, pair, t0:t0 + 512],
                                    ctmp[:])

        def outproj_chunk(tc_i):
            t0 = 512 * tc_i
            # m-blocks in pairs sharing one [128,2,512] staging tile and
            # ONE DMA: halves the sync-queue issue slots at the tail
            for mp in range(4):
                t = oio.tile([P, 2, 512], BF, tag="oio")
                for mh in range(2):
                    pt = ps_b.tile([P, 512], F32, tag="psb")
                    for kt in range(2):
                        nc.tensor.matmul(
                            pt[:], wo_sb[:, kt, 256 * mp + 128 * mh:
                                          256 * mp + 128 * mh + 128],
                            ctx_sb[:, kt, t0:t0 + 512],
                            start=(kt == 0), stop=(kt == 1))
                    nc.vector.tensor_copy(t[:, mh, :], pt[:])
                nc.sync.dma_start(
                    outr[:, 2 * mp:2 * mp + 2, t0:t0 + 512], t[:])

        if causal:
            proj_chunk(0)
            for tc_i in range(3):
                us = attn_chunk(tc_i)
                proj_chunk(tc_i + 1)
                norm_chunk(tc_i, [(0, us[0], us[1]), (1, us[2], us[3])])
                outproj_chunk(tc_i)
            # last chunk: normalize pair0 eagerly (during pair1's
            # attention) so only pair1's norm chain trails the PE
            us = attn_chunk(
                3, eager_norm=lambda u0, u1: norm_chunk(
                    3, [(0, u0, u1)], eager=True))
            norm_chunk(3, [(1, us[2], us[3])], on_pe=True)
            outproj_chunk(3)
        else:
            for tc_i in range(4):
                proj_chunk(tc_i)
            for tc_i in range(4):
                us = attn_chunk(tc_i)
                norm_chunk(tc_i, [(0, us[0], us[1]), (1, us[2], us[3])])
                outproj_chunk(tc_i)

    nc.compile()
    return nc


_CACHE = {}


def _get_nc(causal: bool):
    if causal not in _CACHE:
        _CACHE[causal] = _emit(causal)
    return _CACHE[causal]


def kernel(**inputs):
    x = np.asarray(inputs["x"], dtype=np.float32)
    Wq = np.asarray(inputs["Wq"], dtype=np.float32)
    bq = np.asarray(inputs["bq"], dtype=np.float32)
    Wk = np.asarray(inputs["Wk"], dtype=np.float32)
    bk = np.asarray(inputs["bk"], dtype=np.float32)
    Wv = np.asarray(inputs["Wv"], dtype=np.float32)
    bv = np.asarray(inputs["bv"], dtype=np.float32)
    Wo = np.asarray(inputs["Wo"], dtype=np.float32)
    bo = np.asarray(inputs["bo"], dtype=np.float32)
    causal = bool(int(np.asarray(inputs["enable_causal"])))

    scale = np.float32(1.0 / np.sqrt(HD))
    xTb = [np.ascontiguousarray(x[b].T).astype(NPBF) for b in range(B)]
    cm = np.ascontiguousarray(np.broadcast_to(
        (np.arange(P)[:, None] <= np.arange(P)[None, :])
        .astype(np.float32)[:, None, :], (P, 2, P))).astype(NPBF)

    nc = _get_nc(causal)
    in_maps = []
    for c in range(NC):
        b, g = divmod(c, 4)
        f0 = 256 * g
        m = {"xT": xTb[b],
             "wqT": np.ascontiguousarray(Wq[f0:f0 + 256, :].T).astype(NPBF),
             "wkT": np.ascontiguousarray(
                 (Wk[f0:f0 + 256, :] * scale).T).astype(NPBF),
             "wvT": np.ascontiguousarray(Wv[f0:f0 + 256, :].T).astype(NPBF),
             "woT": np.ascontiguousarray(Wo[:, f0:f0 + 256].T).astype(NPBF),
             "bqc": np.ascontiguousarray(
                 bq[f0:f0 + 256].reshape(2, P).T).astype(np.float32)}
        if causal:
            m["cm"] = cm
        in_maps.append(m)

    global LAST_RESULT
    res = run_bass_kernel_spmd(nc, in_maps, list(range(NC)), trace=TRACE)
    LAST_RESULT = res
    out = np.zeros((B, S, D), dtype=np.float32)
    for c in range(NC):
        b = c // 4
        out[b] += np.asarray(res.results[c]["outT"], dtype=np.float32).T
    out += (bo + bv @ Wo.T)[None, None, :]
    return out


# revision 35
# speedup vs baseline: 1.0568x; 1.0568x over previous
"""Trainium2 Bass kernel for nn_MultiHeadAttention (B=2, S=2048, D=1024, H=16, causal).

Sharding across 8 NeuronCores -- NO on-device collective:
  - Core c owns batch b=c//4 and head-group g=c%4 (4 heads).  Wq/Wk/Wv are
    column-sharded (256 features per core), Wo is row-sharded; each core
    emits a PARTIAL output projection over the full 2048 tokens and the
    host sums the 4 partials per batch at unshard time.  This removes the
    AllGather + barrier that cost ~190us in the token-sharded design.
  - Everything on-chip is bf16 (PSUM accumulation fp32); the softmax
    scale is folded into Wk on the host.  The K bias is dropped (it adds
    a per-query constant to scores, which softmax cancels); the V bias is
    dropped on-device (attention rows sum to 1, so it becomes the
    constant bv @ Wo.T folded into bo on the host); the Q bias is a
    per-partition tensor_scalar add fused into the PSUM->SBUF copy.
  - Pipelined per 512-token chunk tc: project K/V/Q for chunk tc, run
    attention for query chunk tc (keys 0..512*tc+511), project chunk tc+1
    BEFORE normalize/output-projection of tc so the softmax-normalize
    tail never stalls the PE.
  - Attention per head pair (feature block = 128 partitions): score
    matmuls for the two heads are row-tiled (partitions 0:64 / 64:128)
    into one 2-bank PSUM tile so they run concurrently; one wide exp
    covers both heads; ONE flat software pipeline over (pair, block) --
    the ctx matmuls of block b issue after the scores of block b+1,
    across the pair boundary too, so the PE never waits on the scalar
    engine's exp.  Diagonal key blocks are width-trimmed to 512-128*o
    columns; the causal mask restricted to the computed window is the
    same [128,2,128] staircase for every block (beyond 128 columns it is
    all-ones).  ctx accumulates in PSUM across key blocks with a 65th
    stationary V column (= softmax denominator); ctx is copied raw
    (bf16) to SBUF immediately to free the PSUM banks, and the
    normalize chain -- recip = exp(-ln(d)) on scalar, replicated across
    64 partitions by gpsimd partition_broadcast (the PE and DVE stay
    untouched), DVE mult -- runs off the critical path.  Odd heads
    reach partitions 64:128 of ctx_sb via one small SBUF->SBUF DMA per
    (chunk, pair).  On the LAST chunk, pair0 is normalized eagerly
    (during pair1's attention) so only pair1's chain trails the PE.
  - PSUM: ps_a 2x[128,1024] double-buffers score tiles; ps_b 4x[128,512]
    carries ctx accumulators, projection and output-projection tiles in
    one FIFO rotation.
  - All DMAs issue from the sync/scalar hardware DGE queues (gpsimd's
    software DGE adds ~5us trigger latency); the startup-critical wk +
    x-chunk-0 bytes go first on separate queues so the x bulk never
    competes with them.
"""
import numpy as np
import ml_dtypes

import concourse.bass as bass
import concourse.bacc as bacc
import concourse.mybir as mybir
import concourse.tile as tile
from concourse.bass_utils import run_bass_kernel_spmd

B, S, D, H, HD = 2, 2048, 1024, 16, 64
NC = 8
P = 128
F32 = mybir.dt.float32
BF = mybir.dt.bfloat16
NPBF = ml_dtypes.bfloat16

TRACE = False        # set True (e.g. from test.py) to capture an NTFF profile
LAST_RESULT = None   # BassKernelResults of the most recent kernel() call

_ACT_PATCHED = False


def _patch_act_tables():
    """Steer Bacc's act-table-load pass to the combined natural_log+exp
    set so a kernel using both Exp and Ln takes ONE table load instead of
    alternating between table sets (~1.3us per switch on scalar)."""
    global _ACT_PATCHED
    if _ACT_PATCHED:
        return
    import concourse.bacc as _bacc
    _orig = _bacc.get_activation_tables

    def _filtered(arch):
        t = _orig(arch)
        fexp = mybir.ActivationFunctionType.Exp
        fln = mybir.ActivationFunctionType.Ln
        out = {}
        for name, fns in t.items():
            if name != "natural_log_exp_and_others" and (
                    fexp in fns or fln in fns):
                fns = fns - {fexp, fln}
            out[name] = fns
        return out

    _bacc.get_activation_tables = _filtered
    _ACT_PATCHED = True


def _emit(causal: bool):
    nc = bacc.Bacc(trn_type="TRN2", num_devices=NC)
    fexp = mybir.ActivationFunctionType.Exp
    fln = mybir.ActivationFunctionType.Ln
    _patch_act_tables()

    xT = nc.dram_tensor("xT", [D, S], BF, kind="ExternalInput")
    wqT = nc.dram_tensor("wqT", [D, 256], BF, kind="ExternalInput")
    wkT = nc.dram_tensor("wkT", [D, 256], BF, kind="ExternalInput")
    wvT = nc.dram_tensor("wvT", [D, 256], BF, kind="ExternalInput")
    woT = nc.dram_tensor("woT", [256, D], BF, kind="ExternalInput")
    bqc_d = nc.dram_tensor("bqc", [P, 2], F32, kind="ExternalInput")
    if causal:
        cm_d = nc.dram_tensor("cm", [P, 2, P], BF, kind="ExternalInput")
    outT = nc.dram_tensor("outT", [D, S], BF, kind="ExternalOutput")

    with tile.TileContext(nc) as tc, \
         tc.tile_pool(name="const", bufs=1) as const, \
         tc.tile_pool(name="big", bufs=1) as big, \
         tc.tile_pool(name="oio", bufs=3) as oio, \
         tc.tile_pool(name="ex", bufs=8) as ex, \
         tc.tile_pool(name="u", bufs=4) as up, \
         tc.tile_pool(name="sm", bufs=2) as sm, \
         tc.tile_pool(name="ps_a", bufs=2, space="PSUM") as ps_a, \
         tc.tile_pool(name="ps_b", bufs=4, space="PSUM") as ps_b:

        # ---------- constants / inputs ----------
        ones = const.tile([1, P], BF)
        nc.gpsimd.memset(ones[:], 1.0)
        bqc_sb = const.tile([P, 2], F32)
        nc.scalar.dma_start(bqc_sb[:], bqc_d[:])
        if causal:
            cm_sb = const.tile([P, 2, P], BF)
            nc.scalar.dma_start(cm_sb[:], cm_d[:])

        wk_sb = big.tile([P, 8, 256], BF)
        wv_sb = big.tile([P, 8, 256], BF)
        wq_sb = big.tile([P, 8, 256], BF)
        wo_sb = big.tile([P, 2, D], BF)
        xt_sb = big.tile([P, 8, S], BF)
        kt_sb = big.tile([P, 2, S], BF)
        qt_sb = big.tile([P, 2, S], BF)
        v_sb = big.tile([P, 16, 4, 65], BF)
        ctx_sb = big.tile([P, 2, S], BF)
        nc.gpsimd.memset(v_sb[:, :, :, 64:65], 1.0)

        wkr = wkT.rearrange("(o p) f -> p o f", p=P)
        wvr = wvT.rearrange("(o p) f -> p o f", p=P)
        wqr = wqT.rearrange("(o p) f -> p o f", p=P)
        wor = woT.rearrange("(o p) f -> p o f", p=P)
        xr = xT.rearrange("(o p) t -> p o t", p=P)
        outr = outT.rearrange("(o p) t -> p o t", p=P)

        # Input DMAs: the critical path is wk + x chunk 0 (feeds the
        # first K matmuls).  Only sync/scalar issue DMAs (gpsimd DMAs go
        # through the slow software DGE).  scalar: consts then x chunk 0;
        # sync: weights then the bulk of x, so the bulk never competes
        # with the startup-critical bytes.
        nc.sync.dma_start(wk_sb[:], wkr[:])
        nc.sync.dma_start(wq_sb[:], wqr[:])
        nc.sync.dma_start(wv_sb[:], wvr[:])
        for kt in range(8):
            nc.scalar.dma_start(xt_sb[:, kt, 0:512], xr[:, kt, 0:512])
        for kt in range(8):
            nc.sync.dma_start(xt_sb[:, kt, 512:2048], xr[:, kt, 512:2048])
        nc.sync.dma_start(wo_sb[:], wor[:])

        def proj_chunk(tc_i):
            t0 = 512 * tc_i
            # K^T and Q^T: out[feat, tok], feature block fb == head pair.
            # Q bias is a per-partition scalar add fused into the copy.
            # K bias is DROPPED: it adds a per-query constant to every
            # score column, which the softmax normalization cancels.
            for fb in range(2):
                pt = ps_b.tile([P, 512], F32, tag="psb")
                for kt in range(8):
                    nc.tensor.matmul(
                        pt[:], wk_sb[:, kt, 128 * fb:128 * fb + 128],
                        xt_sb[:, kt, t0:t0 + 512],
                        start=(kt == 0), stop=(kt == 7))
                nc.vector.tensor_copy(kt_sb[:, fb, t0:t0 + 512], pt[:])
            for fb in range(2):
                pt = ps_b.tile([P, 512], F32, tag="psb")
                for kt in range(8):
                    nc.tensor.matmul(
                        pt[:], wq_sb[:, kt, 128 * fb:128 * fb + 128],
                        xt_sb[:, kt, t0:t0 + 512],
                        start=(kt == 0), stop=(kt == 7))
                nc.vector.tensor_scalar_add(
                    qt_sb[:, fb, t0:t0 + 512], pt[:], bqc_sb[:, fb:fb + 1])
            # V: out[tok, feat] per 128-token block (65th col pre-set to
            # 1).  V bias is DROPPED: attention rows sum to 1 after
            # normalization, so it shifts the output by the constant
            # bv @ Wo.T which the host folds into bo.
            for tb in range(4):
                jb = 4 * tc_i + tb
                pt = ps_b.tile([P, 512], F32, tag="psb")
                for kt in range(8):
                    nc.tensor.matmul(
                        pt[:, 0:256],
                        xt_sb[:, kt, t0 + 128 * tb:t0 + 128 * tb + 128],
                        wv_sb[:, kt, :], start=(kt == 0), stop=(kt == 7))
                nc.vector.tensor_copy(
                    v_sb[:, jb, :, 0:64],
                    pt[:, 0:256].rearrange("p (h d) -> p h d", h=4))

        def attn_chunk(tc_i, eager_norm=None):
            """Scores+exp+ctx for both head pairs.  One flat software
            pipeline over blocks (pair-major): the ctx matmuls of block b
            issue after the scores of block b+1, across the pair boundary
            too, so the PE never waits on the scalar engine's exp."""
            t0 = 512 * tc_i
            jn = 4 * tc_i + 4 if causal else 16
            ctx = {}
            prev = None
            us = []

            def emit_ctx(pair, pj, pet, pqo, pwid):
                for hh in range(2):
                    nc.tensor.matmul(
                        ctx[pair][hh][0:65, pqo:pqo + pwid],
                        v_sb[:, pj, 2 * pair + hh, :], pet[:, hh, 0:pwid],
                        start=(pj == 0), stop=(pj == jn - 1))
                if pj == jn - 1:
                    for hh in range(2):
                        u = up.tile([65, 512], BF, tag="u")
                        nc.vector.tensor_copy(u[:], ctx[pair][hh][0:65, :])
                        us.append(u)
                    if eager_norm is not None and pair == 0:
                        eager_norm(us[0], us[1])

            for pair in range(2):
                c0 = ps_b.tile([P, 512], F32, tag="psb")
                c1 = ps_b.tile([P, 512], F32, tag="psb")
                ctx[pair] = (c0, c1)
                for j in range(jn):
                    o_ = j - 4 * tc_i if causal else -1
                    qo = 0 if o_ < 0 else 128 * o_
                    wid = 512 - qo
                    sc = ps_a.tile([P, 1024], F32, tag="psa")
                    for hh in range(2):
                        nc.tensor.matmul(
                            sc[:, 512 * hh:512 * hh + wid],
                            kt_sb[64 * hh:64 * hh + 64, pair,
                                  128 * j:128 * j + 128],
                            qt_sb[64 * hh:64 * hh + 64, pair,
                                  t0 + qo:t0 + qo + wid],
                            start=True, stop=True)
                    et = ex.tile([P, 2, 512], BF, tag="exp")
                    if wid == 512:
                        nc.scalar.activation(et[:, :, :], sc[:, :], fexp)
                    else:
                        nc.scalar.activation(
                            et[:, :, 0:wid],
                            sc[:].rearrange("p (s n) -> p s n", s=2)
                            [:, :, 0:wid], fexp)
                    if o_ >= 0:
                        nc.vector.tensor_tensor(
                            et[:, :, 0:P], et[:, :, 0:P], cm_sb[:],
                            mybir.AluOpType.mult)
                    if prev is not None:
                        emit_ctx(*prev)
                    prev = (pair, j, et, qo, wid)
            emit_ctx(*prev)
            return us

        def norm_chunk(tc_i, pairs, on_pe=False):
            """recip = exp(-ln(denominator)) for both heads of a pair into
            one [1,1024] tile; ONE rank-1 matmul replicates it to a
            [64,1024] PSUM region that the DVE mults read directly (no
            PSUM->SBUF copy).  Even heads multiply straight into
            ctx_sb[0:64]; odd heads go via a [64,2,512] staging tile and
            ONE SBUF->SBUF DMA per chunk to partitions 64:128."""
            t0 = 512 * tc_i
            for pair, u0, u1 in pairs:
                rcp2 = sm.tile([1, 1024], BF, tag="rcp")
                lnd0 = sm.tile([1, 512], F32, tag="lnd")
                nc.scalar.activation(lnd0[:], u0[64:65, 0:512], fln)
                nc.scalar.activation(rcp2[0:1, 0:512], lnd0[:], fexp,
                                     scale=-1.0)
                lnd1 = sm.tile([1, 512], F32, tag="lnd")
                nc.scalar.activation(lnd1[:], u1[64:65, 0:512], fln)
                nc.scalar.activation(rcp2[0:1, 512:1024], lnd1[:], fexp,
                                     scale=-1.0)
                if on_pe:
                    rep_ps = ps_b.tile([P, 512], F32, tag="psb")
                    nc.tensor.matmul(rep_ps[0:64, :], ones[0:1, 0:64],
                                     rcp2[0:1, 0:512], start=True, stop=True)
                    rep_ps2 = ps_b.tile([P, 512], F32, tag="psb")
                    nc.tensor.matmul(rep_ps2[0:64, :], ones[0:1, 0:64],
                                     rcp2[0:1, 512:1024],
                                     start=True, stop=True)
                    r0, r1 = rep_ps[0:64, :], rep_ps2[0:64, :]
                else:
                    rep = sm.tile([64, 1024], BF, tag="rep")
                    nc.gpsimd.partition_broadcast(rep[:], rcp2[0:1, :])
                    r0, r1 = rep[:, 0:512], rep[:, 512:1024]
                nc.vector.tensor_tensor(
                    ctx_sb[0:64, pair, t0:t0 + 512], u0[0:64, :],
                    r0, mybir.AluOpType.mult)
                ctmp = sm.tile([64, 512], BF, tag="ctmp")
                nc.vector.tensor_tensor(
                    ctmp[:], u1[0:64, :],
                    r1, mybir.AluOpType.mult)
                # per-pair DMA: pair0's transfer overlaps pair1's norm
                nc.scalar.dma_start(ctx_sb[64:128, pair, t0:t0 + 512],
                                    ctmp[:])

        def outproj_chunk(tc_i):
            t0 = 512 * tc_i
            # m-blocks in pairs sharing one [128,2,512] staging tile and
            # ONE DMA: halves the sync-queue issue slots at the tail
            for mp in range(4):
                t = oio.tile([P, 2, 512], BF, tag="oio")
                for mh in range(2):
                    pt = ps_b.tile([P, 512], F32, tag="psb")
                    for kt in range(2):
                        nc.tensor.matmul(
                            pt[:], wo_sb[:, kt, 256 * mp + 128 * mh:
                                          256 * mp + 128 * mh + 128],
                            ctx_sb[:, kt, t0:t0 + 512],
                            start=(kt == 0), stop=(kt == 1))
                    nc.vector.tensor_copy(t[:, mh, :], pt[:])
                nc.sync.dma_start(
                    outr[:, 2 * mp:2 * mp + 2, t0:t0 + 512], t[:])

        if causal:
            proj_chunk(0)
            for tc_i in range(3):
                us = attn_chunk(tc_i)
                proj_chunk(tc_i + 1)
                norm_chunk(tc_i, [(0, us[0], us[1]), (1, us[2], us[3])])
                outproj_chunk(tc_i)
            # last chunk: normalize pair0 eagerly (during pair1's
            # attention) so only pair1's norm chain trails the PE
            us = attn_chunk(3)
            norm_chunk(3, [(0, us[0], us[1]), (1, us[2], us[3])],
                       on_pe=True)
            outproj_chunk(3)
        else:
            for tc_i in range(4):
                proj_chunk(tc_i)
            for tc_i in range(4):
                us = attn_chunk(tc_i)
                norm_chunk(tc_i, [(0, us[0], us[1]), (1, us[2], us[3])])
                outproj_chunk(tc_i)

    nc.compile()
    return nc


_CACHE = {}


def _get_nc(causal: bool):
    if causal not in _CACHE:
        _CACHE[causal] = _emit(causal)
    return _CACHE[causal]


def kernel(**inputs):
    x = np.asarray(inputs["x"], dtype=np.float32)
    Wq = np.asarray(inputs["Wq"], dtype=np.float32)
    bq = np.asarray(inputs["bq"], dtype=np.float32)
    Wk = np.asarray(inputs["Wk"], dtype=np.float32)
    bk = np.asarray(inputs["bk"], dtype=np.float32)
    Wv = np.asarray(inputs["Wv"], dtype=np.float32)
    bv = np.asarray(inputs["bv"], dtype=np.float32)
    Wo = np.asarray(inputs["Wo"], dtype=np.float32)
    bo = np.asarray(inputs["bo"], dtype=np.float32)
    causal = bool(int(np.asarray(inputs["enable_causal"])))

    scale = np.float32(1.0 / np.sqrt(HD))
    xTb = [np.ascontiguousarray(x[b].T).astype(NPBF) for b in range(B)]
    cm = np.ascontiguousarray(np.broadcast_to(
        (np.arange(P)[:, None] <= np.arange(P)[None, :])
        .astype(np.float32)[:, None, :], (P, 2, P))).astype(NPBF)

    nc = _get_nc(causal)
    in_maps = []
    for c in range(NC):
        b, g = divmod(c, 4)
        f0 = 256 * g
        m = {"xT": xTb[b],
             "wqT": np.ascontiguousarray(Wq[f0:f0 + 256, :].T).astype(NPBF),
             "wkT": np.ascontiguousarray(
                 (Wk[f0:f0 + 256, :] * scale).T).astype(NPBF),
             "wvT": np.ascontiguousarray(Wv[f0:f0 + 256, :].T).astype(NPBF),
             "woT": np.ascontiguousarray(Wo[:, f0:f0 + 256].T).astype(NPBF),
             "bqc": np.ascontiguousarray(
                 bq[f0:f0 + 256].reshape(2, P).T).astype(np.float32)}
        if causal:
            m["cm"] = cm
        in_maps.append(m)

    global LAST_RESULT
    res = run_bass_kernel_spmd(nc, in_maps, list(range(NC)), trace=TRACE)
    LAST_RESULT = res
    out = np.zeros((B, S, D), dtype=np.float32)
    for c in range(NC):
        b = c // 4
        out[b] += np.asarray(res.results[c]["outT"], dtype=np.float32).T
    out += (bo + bv @ Wo.T)[None, None, :]
    return out


# revision 36
# speedup vs baseline: 1.0738x; 1.0160x over previous
"""Trainium2 Bass kernel for nn_MultiHeadAttention (B=2, S=2048, D=1024, H=16, causal).

Sharding across 8 NeuronCores -- NO on-device collective:
  - Core c owns batch b=c//4 and head-group g=c%4 (4 heads).  Wq/Wk/Wv are
    column-sharded (256 features per core), Wo is row-sharded; each core
    emits a PARTIAL output projection over the full 2048 tokens and the
    host sums the 4 partials per batch at unshard time.  This removes the
    AllGather + barrier that cost ~190us in the token-sharded design.
  - Everything on-chip is bf16 (PSUM accumulation fp32); the softmax
    scale is folded into Wk on the host.  The K bias is dropped (it adds
    a per-query constant to scores, which softmax cancels); the V bias is
    dropped on-device (attention rows sum to 1, so it becomes the
    constant bv @ Wo.T folded into bo on the host); the Q bias is a
    per-partition tensor_scalar add fused into the PSUM->SBUF copy.
  - Pipelined per 512-token chunk tc: project K/V/Q for chunk tc, run
    attention for query chunk tc (keys 0..512*tc+511), project chunk tc+1
    BEFORE normalize/output-projection of tc so the softmax-normalize
    tail never stalls the PE.
  - Attention per head pair (feature block = 128 partitions): score
    matmuls for the two heads are row-tiled (partitions 0:64 / 64:128)
    into one 2-bank PSUM tile so they run concurrently; one wide exp
    covers both heads; ONE flat software pipeline over (pair, block) --
    the ctx matmuls of block b issue after the scores of block b+1,
    across the pair boundary too, so the PE never waits on the scalar
    engine's exp.  Diagonal key blocks are width-trimmed to 512-128*o
    columns; the causal mask restricted to the computed window is the
    same [128,2,128] staircase for every block (beyond 128 columns it is
    all-ones).  ctx accumulates in PSUM across key blocks with a 65th
    stationary V column (= softmax denominator); ctx is copied raw
    (bf16) to SBUF immediately to free the PSUM banks, and the
    normalize chain -- recip = exp(-ln(d)) on scalar, replicated across
    64 partitions by gpsimd partition_broadcast mid-run (by bf16 rank-1
    PE matmuls on the last chunk, where the PE is otherwise idle and
    the ~1.7us gpsimd op would sit on the critical path), DVE mult --
    runs off the critical path.  Odd heads reach partitions 64:128 of
    ctx_sb via one small SBUF->SBUF DMA per (chunk, pair).
  - PSUM: ps_a 2x[128,1024] double-buffers score tiles; ps_b 4x[128,512]
    carries ctx accumulators, projection and output-projection tiles in
    one FIFO rotation.
  - All DMAs issue from the sync/scalar hardware DGE queues (gpsimd's
    software DGE adds ~5us trigger latency); the startup-critical wk +
    x-chunk-0 bytes go first on separate queues so the x bulk never
    competes with them.
"""
import numpy as np
import ml_dtypes

import concourse.bass as bass
import concourse.bacc as bacc
import concourse.mybir as mybir
import concourse.tile as tile
from concourse.bass_utils import run_bass_kernel_spmd

B, S, D, H, HD = 2, 2048, 1024, 16, 64
NC = 8
P = 128
F32 = mybir.dt.float32
BF = mybir.dt.bfloat16
NPBF = ml_dtypes.bfloat16

TRACE = False        # set True (e.g. from test.py) to capture an NTFF profile
LAST_RESULT = None   # BassKernelResults of the most recent kernel() call

_ACT_PATCHED = False


def _patch_act_tables():
    """Steer Bacc's act-table-load pass to the combined natural_log+exp
    set so a kernel using both Exp and Ln takes ONE table load instead of
    alternating between table sets (~1.3us per switch on scalar)."""
    global _ACT_PATCHED
    if _ACT_PATCHED:
        return
    import concourse.bacc as _bacc
    _orig = _bacc.get_activation_tables

    def _filtered(arch):
        t = _orig(arch)
        fexp = mybir.ActivationFunctionType.Exp
        fln = mybir.ActivationFunctionType.Ln
        out = {}
        for name, fns in t.items():
            if name != "natural_log_exp_and_others" and (
                    fexp in fns or fln in fns):
                fns = fns - {fexp, fln}
            out[name] = fns
        return out

    _bacc.get_activation_tables = _filtered
    _ACT_PATCHED = True


def _emit(causal: bool):
    nc = bacc.Bacc(trn_type="TRN2", num_devices=NC)
    fexp = mybir.ActivationFunctionType.Exp
    fln = mybir.ActivationFunctionType.Ln
    _patch_act_tables()

    xT = nc.dram_tensor("xT", [D, S], BF, kind="ExternalInput")
    wqT = nc.dram_tensor("wqT", [D, 256], BF, kind="ExternalInput")
    wkT = nc.dram_tensor("wkT", [D, 256], BF, kind="ExternalInput")
    wvT = nc.dram_tensor("wvT", [D, 256], BF, kind="ExternalInput")
    woT = nc.dram_tensor("woT", [256, D], BF, kind="ExternalInput")
    bqc_d = nc.dram_tensor("bqc", [P, 2], F32, kind="ExternalInput")
    if causal:
        cm_d = nc.dram_tensor("cm", [P, 2, P], BF, kind="ExternalInput")
    outT = nc.dram_tensor("outT", [D, S], BF, kind="ExternalOutput")

    with tile.TileContext(nc) as tc, \
         tc.tile_pool(name="const", bufs=1) as const, \
         tc.tile_pool(name="big", bufs=1) as big, \
         tc.tile_pool(name="oio", bufs=3) as oio, \
         tc.tile_pool(name="ex", bufs=8) as ex, \
         tc.tile_pool(name="u", bufs=4) as up, \
         tc.tile_pool(name="sm", bufs=2) as sm, \
         tc.tile_pool(name="ps_a", bufs=2, space="PSUM") as ps_a, \
         tc.tile_pool(name="ps_b", bufs=4, space="PSUM") as ps_b:

        # ---------- constants / inputs ----------
        ones = const.tile([1, P], BF)
        nc.gpsimd.memset(ones[:], 1.0)
        bqc_sb = const.tile([P, 2], F32)
        nc.scalar.dma_start(bqc_sb[:], bqc_d[:])
        if causal:
            cm_sb = const.tile([P, 2, P], BF)
            nc.scalar.dma_start(cm_sb[:], cm_d[:])

        wk_sb = big.tile([P, 8, 256], BF)
        wv_sb = big.tile([P, 8, 256], BF)
        wq_sb = big.tile([P, 8, 256], BF)
        wo_sb = big.tile([P, 2, D], BF)
        xt_sb = big.tile([P, 8, S], BF)
        kt_sb = big.tile([P, 2, S], BF)
        qt_sb = big.tile([P, 2, S], BF)
        v_sb = big.tile([P, 16, 4, 65], BF)
        ctx_sb = big.tile([P, 2, S], BF)
        nc.gpsimd.memset(v_sb[:, :, :, 64:65], 1.0)

        wkr = wkT.rearrange("(o p) f -> p o f", p=P)
        wvr = wvT.rearrange("(o p) f -> p o f", p=P)
        wqr = wqT.rearrange("(o p) f -> p o f", p=P)
        wor = woT.rearrange("(o p) f -> p o f", p=P)
        xr = xT.rearrange("(o p) t -> p o t", p=P)
        outr = outT.rearrange("(o p) t -> p o t", p=P)

        # Input DMAs: the critical path is wk + x chunk 0 (feeds the
        # first K matmuls).  Only sync/scalar issue DMAs (gpsimd DMAs go
        # through the slow software DGE).  scalar: consts then x chunk 0;
        # sync: weights then the bulk of x, so the bulk never competes
        # with the startup-critical bytes.
        nc.sync.dma_start(wk_sb[:], wkr[:])
        nc.sync.dma_start(wq_sb[:], wqr[:])
        nc.sync.dma_start(wv_sb[:], wvr[:])
        for kt in range(8):
            nc.scalar.dma_start(xt_sb[:, kt, 0:512], xr[:, kt, 0:512])
        for kt in range(8):
            nc.sync.dma_start(xt_sb[:, kt, 512:2048], xr[:, kt, 512:2048])
        nc.sync.dma_start(wo_sb[:], wor[:])

        def proj_chunk(tc_i):
            t0 = 512 * tc_i
            # K^T and Q^T: out[feat, tok], feature block fb == head pair.
            # Q bias is a per-partition scalar add fused into the copy.
            # K bias is DROPPED: it adds a per-query constant to every
            # score column, which the softmax normalization cancels.
            for fb in range(2):
                pt = ps_b.tile([P, 512], F32, tag="psb")
                for kt in range(8):
                    nc.tensor.matmul(
                        pt[:], wk_sb[:, kt, 128 * fb:128 * fb + 128],
                        xt_sb[:, kt, t0:t0 + 512],
                        start=(kt == 0), stop=(kt == 7))
                nc.vector.tensor_copy(kt_sb[:, fb, t0:t0 + 512], pt[:])
            for fb in range(2):
                pt = ps_b.tile([P, 512], F32, tag="psb")
                for kt in range(8):
                    nc.tensor.matmul(
                        pt[:], wq_sb[:, kt, 128 * fb:128 * fb + 128],
                        xt_sb[:, kt, t0:t0 + 512],
                        start=(kt == 0), stop=(kt == 7))
                nc.vector.tensor_scalar_add(
                    qt_sb[:, fb, t0:t0 + 512], pt[:], bqc_sb[:, fb:fb + 1])
            # V: out[tok, feat] per 128-token block (65th col pre-set to
            # 1).  V bias is DROPPED: attention rows sum to 1 after
            # normalization, so it shifts the output by the constant
            # bv @ Wo.T which the host folds into bo.
            for tb in range(4):
                jb = 4 * tc_i + tb
                pt = ps_b.tile([P, 512], F32, tag="psb")
                for kt in range(8):
                    nc.tensor.matmul(
                        pt[:, 0:256],
                        xt_sb[:, kt, t0 + 128 * tb:t0 + 128 * tb + 128],
                        wv_sb[:, kt, :], start=(kt == 0), stop=(kt == 7))
                nc.vector.tensor_copy(
                    v_sb[:, jb, :, 0:64],
                    pt[:, 0:256].rearrange("p (h d) -> p h d", h=4))

        def attn_chunk(tc_i, eager_norm=None):
            """Scores+exp+ctx for both head pairs.  One flat software
            pipeline over blocks (pair-major): the ctx matmuls of block b
            issue after the scores of block b+1, across the pair boundary
            too, so the PE never waits on the scalar engine's exp."""
            t0 = 512 * tc_i
            jn = 4 * tc_i + 4 if causal else 16
            ctx = {}
            prev = None
            us = []

            def emit_ctx(pair, pj, pet, pqo, pwid):
                for hh in range(2):
                    nc.tensor.matmul(
                        ctx[pair][hh][0:65, pqo:pqo + pwid],
                        v_sb[:, pj, 2 * pair + hh, :], pet[:, hh, 0:pwid],
                        start=(pj == 0), stop=(pj == jn - 1))
                if pj == jn - 1:
                    for hh in range(2):
                        u = up.tile([65, 512], BF, tag="u")
                        nc.vector.tensor_copy(u[:], ctx[pair][hh][0:65, :])
                        us.append(u)
                    if eager_norm is not None and pair == 0:
                        eager_norm(us[0], us[1])

            for pair in range(2):
                c0 = ps_b.tile([P, 512], F32, tag="psb")
                c1 = ps_b.tile([P, 512], F32, tag="psb")
                ctx[pair] = (c0, c1)
                for j in range(jn):
                    o_ = j - 4 * tc_i if causal else -1
                    qo = 0 if o_ < 0 else 128 * o_
                    wid = 512 - qo
                    sc = ps_a.tile([P, 1024], F32, tag="psa")
                    for hh in range(2):
                        nc.tensor.matmul(
                            sc[:, 512 * hh:512 * hh + wid],
                            kt_sb[64 * hh:64 * hh + 64, pair,
                                  128 * j:128 * j + 128],
                            qt_sb[64 * hh:64 * hh + 64, pair,
                                  t0 + qo:t0 + qo + wid],
                            start=True, stop=True)
                    et = ex.tile([P, 2, 512], BF, tag="exp")
                    if wid == 512:
                        nc.scalar.activation(et[:, :, :], sc[:, :], fexp)
                    else:
                        nc.scalar.activation(
                            et[:, :, 0:wid],
                            sc[:].rearrange("p (s n) -> p s n", s=2)
                            [:, :, 0:wid], fexp)
                    if o_ >= 0:
                        nc.vector.tensor_tensor(
                            et[:, :, 0:P], et[:, :, 0:P], cm_sb[:],
                            mybir.AluOpType.mult)
                    if prev is not None:
                        emit_ctx(*prev)
                    prev = (pair, j, et, qo, wid)
            emit_ctx(*prev)
            return us

        def norm_chunk(tc_i, pairs, on_pe=False):
            """recip = exp(-ln(denominator)) for both heads of a pair into
            one [1,1024] tile; ONE rank-1 matmul replicates it to a
            [64,1024] PSUM region that the DVE mults read directly (no
            PSUM->SBUF copy).  Even heads multiply straight into
            ctx_sb[0:64]; odd heads go via a [64,2,512] staging tile and
            ONE SBUF->SBUF DMA per chunk to partitions 64:128."""
            t0 = 512 * tc_i
            for pair, u0, u1 in pairs:
                rcp2 = sm.tile([1, 1024], BF, tag="rcp")
                lnd0 = sm.tile([1, 512], F32, tag="lnd")
                nc.scalar.activation(lnd0[:], u0[64:65, 0:512], fln)
                nc.scalar.activation(rcp2[0:1, 0:512], lnd0[:], fexp,
                                     scale=-1.0)
                lnd1 = sm.tile([1, 512], F32, tag="lnd")
                nc.scalar.activation(lnd1[:], u1[64:65, 0:512], fln)
                nc.scalar.activation(rcp2[0:1, 512:1024], lnd1[:], fexp,
                                     scale=-1.0)
                if on_pe:
                    rep_ps = ps_b.tile([P, 512], F32, tag="psb")
                    nc.tensor.matmul(rep_ps[0:64, :], ones[0:1, 0:64],
                                     rcp2[0:1, 0:512], start=True, stop=True)
                    rep_ps2 = ps_b.tile([P, 512], F32, tag="psb")
                    nc.tensor.matmul(rep_ps2[0:64, :], ones[0:1, 0:64],
                                     rcp2[0:1, 512:1024],
                                     start=True, stop=True)
                    r0, r1 = rep_ps[0:64, :], rep_ps2[0:64, :]
                else:
                    rep = sm.tile([64, 1024], BF, tag="rep")
                    nc.gpsimd.partition_broadcast(rep[:], rcp2[0:1, :])
                    r0, r1 = rep[:, 0:512], rep[:, 512:1024]
                nc.vector.tensor_tensor(
                    ctx_sb[0:64, pair, t0:t0 + 512], u0[0:64, :],
                    r0, mybir.AluOpType.mult)
                ctmp = sm.tile([64, 512], BF, tag="ctmp")
                nc.vector.tensor_tensor(
                    ctmp[:], u1[0:64, :],
                    r1, mybir.AluOpType.mult)
                # per-pair DMA: pair0's transfer overlaps pair1's norm
                nc.scalar.dma_start(ctx_sb[64:128, pair, t0:t0 + 512],
                                    ctmp[:])

        def outproj_chunk(tc_i):
            t0 = 512 * tc_i
            # m-blocks in pairs sharing one [128,2,512] staging tile and
            # ONE DMA: halves the sync-queue issue slots at the tail
            for mp in range(4):
                t = oio.tile([P, 2, 512], BF, tag="oio")
                for mh in range(2):
                    pt = ps_b.tile([P, 512], F32, tag="psb")
                    for kt in range(2):
                        nc.tensor.matmul(
                            pt[:], wo_sb[:, kt, 256 * mp + 128 * mh:
                                          256 * mp + 128 * mh + 128],
                            ctx_sb[:, kt, t0:t0 + 512],
                            start=(kt == 0), stop=(kt == 1))
                    nc.vector.tensor_copy(t[:, mh, :], pt[:])
                nc.sync.dma_start(
                    outr[:, 2 * mp:2 * mp + 2, t0:t0 + 512], t[:])

        if causal:
            proj_chunk(0)
            for tc_i in range(3):
                us = attn_chunk(tc_i)
                proj_chunk(tc_i + 1)
                norm_chunk(tc_i, [(0, us[0], us[1]), (1, us[2], us[3])])
                outproj_chunk(tc_i)
            # last chunk: normalize pair0 eagerly (during pair1's
            # attention) so only pair1's norm chain trails the PE
            us = attn_chunk(3)
            norm_chunk(3, [(0, us[0], us[1]), (1, us[2], us[3])],
                       on_pe=True)
            outproj_chunk(3)
        else:
            for tc_i in range(4):
                proj_chunk(tc_i)
            for tc_i in range(4):
                us = attn_chunk(tc_i)
                norm_chunk(tc_i, [(0, us[0], us[1]), (1, us[2], us[3])])
                outproj_chunk(tc_i)

    nc.compile()
    return nc


_CACHE = {}


def _get_nc(causal: bool):
    if causal not in _CACHE:
        _CACHE[causal] = _emit(causal)
    return _CACHE[causal]


def kernel(**inputs):
    x = np.asarray(inputs["x"], dtype=np.float32)
    Wq = np.asarray(inputs["Wq"], dtype=np.float32)
    bq = np.asarray(inputs["bq"], dtype=np.float32)
    Wk = np.asarray(inputs["Wk"], dtype=np.float32)
    bk = np.asarray(inputs["bk"], dtype=np.float32)
    Wv = np.asarray(inputs["Wv"], dtype=np.float32)
    bv = np.asarray(inputs["bv"], dtype=np.float32)
    Wo = np.asarray(inputs["Wo"], dtype=np.float32)
    bo = np.asarray(inputs["bo"], dtype=np.float32)
    causal = bool(int(np.asarray(inputs["enable_causal"])))

    scale = np.float32(1.0 / np.sqrt(HD))
    xTb = [np.ascontiguousarray(x[b].T).astype(NPBF) for b in range(B)]
    cm = np.ascontiguousarray(np.broadcast_to(
        (np.arange(P)[:, None] <= np.arange(P)[None, :])
        .astype(np.float32)[:, None, :], (P, 2, P))).astype(NPBF)

    nc = _get_nc(causal)
    in_maps = []
    for c in range(NC):
        b, g = divmod(c, 4)
        f0 = 256 * g
        m = {"xT": xTb[b],
             "wqT": np.ascontiguousarray(Wq[f0:f0 + 256, :].T).astype(NPBF),
             "wkT": np.ascontiguousarray(
                 (Wk[f0:f0 + 256, :] * scale).T).astype(NPBF),
             "wvT": np.ascontiguousarray(Wv[f0:f0 + 256, :].T).astype(NPBF),
             "woT": np.ascontiguousarray(Wo[:, f0:f0 + 256].T).astype(NPBF),
             "bqc": np.ascontiguousarray(
                 bq[f0:f0 + 256].reshape(2, P).T).astype(np.float32)}
        if causal:
            m["cm"] = cm
        in_maps.append(m)

    global LAST_RESULT
    res = run_bass_kernel_spmd(nc, in_maps, list(range(NC)), trace=TRACE)
    LAST_RESULT = res
    out = np.zeros((B, S, D), dtype=np.float32)
    for c in range(NC):
        b = c // 4
        out[b] += np.asarray(res.results[c]["outT"], dtype=np.float32).T
    out += (bo + bv @ Wo.T)[None, None, :]
    return out
